# revision 19
# baseline (speedup 1.0000x reference)
"""Trainium2 Bass kernel for nn_DrugRank (GNN message passing), 8 NeuronCores.

Architecture (v2 — dense-block aggregation, pruned bio branch):

  - The reference consumes only row -1 (node 49999) of the bio GCN stack, so
    the 800k-edge bio branch is dead code except the 2-hop in-neighborhood of
    that node: ~16 L2 edges and ~280 L1 edges. Host prep extracts that
    neighborhood; the device does a handful of matmuls, replicated per core.
  - cll graph (3451 nodes, 55216 edges, 4 GCN layers): dst-node sharded,
    512 nodes (4 blocks of 128) per core. All GCN normalization (symmetric
    deg^-1/2 + self loops) is folded host-side into dense per-(src-chunk,
    dst-block) adjacency tiles Q[s,d] = dinv_s*dinv_d*cnt(s->d) +
    dinv_d^2*[s==d]; aggregation = relu(Q^T h + b) via 27x4 PSUM-accumulated
    128x128x200 matmuls per layer. No dma_gather anywhere in the cll path;
    identical work on every core (no stragglers).
  - Per layer: transform own slice (h = c @ W via PE transposes), AllGather
    the 200KB slice, reload the 1.35MB table, aggregate. 4 AllGathers + one
    1000-float AllReduce (dense-1 partials) are the only collectives.
  - mol branch + fusion head replicated on every core; head weights stream
    during the cll chain. Dense-1 (10353x1000) row-sharded with the node
    sharding, reduced by the AllReduce.
"""

import numpy as np

import concourse.bacc as bacc
import concourse.bass as bass
import concourse.mybir as mybir
import concourse.tile as tile
from concourse.bass_utils import run_bass_kernel_spmd

NCORES = 8
P = 128
F = 200

CLL_N, CLL_E, CLL_PAD, CLL_NPC = 3451, 55216, 4096, 512
CLL_NBLK = CLL_NPC // P                 # 4 dst blocks / core
CLL_NCH = 27                            # src chunks with real nodes
N_BIO = 50000
BIO_S2, BIO_S1 = 768, 128               # padded bio 2-hop sets
BIO_NCH = BIO_S2 // P                   # 6
MOL_N, MOL_E = 64, 128

f32 = mybir.dt.float32
f16 = mybir.dt.float16
i16 = mybir.dt.int16
RELU = mybir.ActivationFunctionType.Relu
COPY = mybir.ActivationFunctionType.Copy
EQ = mybir.AluOpType.is_equal
MUL = mybir.AluOpType.mult
ADD = mybir.AluOpType.add


# ---------------------------------------------------------------- host prep

def _pack_idx16(flat):
    n = len(flat)
    a16 = np.asarray(flat, np.int16).reshape(n // 16, 16).T
    return np.ascontiguousarray(np.tile(a16, (8, 1)))


def _pack_slots(flat, dtype=np.float16):
    n = len(flat)
    return np.ascontiguousarray(
        np.asarray(flat, np.float64).astype(dtype).reshape(n // P, P).T)


def _col(v):
    return np.ascontiguousarray(np.asarray(v, np.float32).reshape(-1, 1))


def _rep(v, rows=P):
    return np.ascontiguousarray(
        np.tile(np.asarray(v, np.float32).reshape(1, -1), (rows, 1)))


def _btile(v, p, n):
    return np.ascontiguousarray(np.asarray(v, np.float32).reshape(n, p).T)


def _cll_q(edge, dinv):
    """Dense normalized adjacency, [CLL_PAD, CLL_PAD] f32."""
    src = edge[0].astype(np.int64)
    dst = edge[1].astype(np.int64)
    q = np.zeros((CLL_PAD, CLL_PAD), np.float32)
    np.add.at(q, (src, dst), (dinv[src] * dinv[dst]).astype(np.float32))
    di = np.arange(CLL_N)
    q[di, di] += (dinv[:CLL_N] * dinv[:CLL_N]).astype(np.float32)
    return q


def _bio_prune(edge, x_bio):
    """2-hop in-neighborhood of node N_BIO-1 -> (xbT_sel, Qb1_pk, Qb2)."""
    src = edge[0].astype(np.int64)
    dst = edge[1].astype(np.int64)
    deg = np.bincount(dst, minlength=N_BIO).astype(np.float64) + 1.0
    dinv = 1.0 / np.sqrt(deg)
    tgt = N_BIO - 1

    m2 = dst == tgt
    s1 = np.unique(np.concatenate([src[m2], [tgt]]))
    assert len(s1) <= BIO_S1, len(s1)
    pos1 = np.full(N_BIO, -1, np.int64)
    pos1[s1] = np.arange(len(s1))

    m1 = pos1[dst] >= 0
    e1s, e1d = src[m1], dst[m1]
    s2 = np.unique(np.concatenate([e1s, s1]))
    assert len(s2) <= BIO_S2, len(s2)
    pos2 = np.full(N_BIO, -1, np.int64)
    pos2[s2] = np.arange(len(s2))

    q1 = np.zeros((BIO_S2, BIO_S1), np.float32)
    np.add.at(q1, (pos2[e1s], pos1[e1d]),
              (dinv[e1s] * dinv[e1d]).astype(np.float32))
    q1[pos2[s1], pos1[s1]] += (dinv[s1] * dinv[s1]).astype(np.float32)

    q2 = np.zeros((BIO_S1, 1), np.float32)
    np.add.at(q2, (pos1[src[m2]], 0),
              (dinv[src[m2]] * dinv[tgt]).astype(np.float32))
    q2[pos1[tgt], 0] += np.float32(dinv[tgt] * dinv[tgt])

    xsel = np.zeros((BIO_S2, 256), np.float32)
    xsel[:len(s2)] = x_bio[s2]
    xbT = np.ascontiguousarray(xsel.T).astype(np.float16)      # [256, 768]
    # Qb1 packed [128, 6*128]: [p, s*128+d] = q1[s*128+p, d]
    q1pk = np.ascontiguousarray(
        q1.reshape(BIO_NCH, P, BIO_S1).transpose(1, 0, 2)
        .reshape(P, BIO_NCH * BIO_S1)).astype(np.float16)
    return xbT, q1pk, q2.astype(np.float16)


def prep_inputs(inp):
    meta = {}
    # ---- cll Q tiles ----
    dst = inp["edge_cll"][1].astype(np.int64)
    deg = np.bincount(dst, minlength=CLL_N).astype(np.float64) + 1.0
    dinv = np.zeros(CLL_PAD, np.float64)
    dinv[:CLL_N] = 1.0 / np.sqrt(deg)
    q = _cll_q(inp["edge_cll"], dinv)

    xcT = np.zeros((512, CLL_PAD), np.float32)
    xcT[:, :CLL_N] = inp["x_cll"].T

    # W1c regrouped: rows (node*3+ch) -> per core [128, 12*1000] f16,
    # col-block j = ch*4+blk, rows = local node p of that block.
    w1c = np.asarray(inp["Wl1c"], np.float32)                  # [10353, 1000]
    w1c_n = np.zeros((CLL_PAD, 3, 1000), np.float32)
    w1c_n[:CLL_N] = w1c.reshape(CLL_N, 3, 1000)

    xbT_sel, q1pk, q2 = _bio_prune(inp["edge_bio"], np.asarray(inp["x_bio"]))

    mol_s = inp["edge_mol"][0].astype(np.int64)
    mol_d = inp["edge_mol"][1].astype(np.int64)
    order = np.argsort(mol_d, kind="stable")
    mol_idx = _pack_idx16(mol_s[order])
    mol_slot = _pack_slots(mol_d[order].astype(np.float64), np.float32)

    iota = np.tile(np.arange(P, dtype=np.float32), (P, 1))
    ident = np.eye(P, dtype=np.float32)

    wc1 = np.asarray(inp["Wc1"], np.float32)                   # [512, 200]
    shared = {
        "Wc1": np.ascontiguousarray(
            wc1.reshape(4, P, F).transpose(1, 0, 2)
            .reshape(P, 4 * F)).astype(np.float16),
        "Wc2": np.asarray(inp["Wc2"], np.float16),
        "Wc3": np.asarray(inp["Wc3"], np.float16),
        "Wc4": np.asarray(inp["Wc4"], np.float16),
        "bc1_rep": _rep(inp["bc1"]), "bc2_rep": _rep(inp["bc2"]),
        "bc3_rep": _rep(inp["bc3"]), "bc4_rep": _rep(inp["bc4"]),
        "xbioT": xbT_sel, "Qb1": q1pk, "Qb2": q2,
        "Wb1": np.asarray(inp["Wb1"], np.float16),
        "Wb2": np.asarray(inp["Wb2"], np.float16),
        "bb1_rep": _rep(inp["bb1"]),
        "bb2_row": np.ascontiguousarray(
            np.asarray(inp["bb2"], np.float32).reshape(1, -1)),
        "x_mol": np.asarray(inp["x_mol"], np.float32),
        "xmolT": np.ascontiguousarray(inp["x_mol"].T.astype(np.float32)),
        "mol_idx": mol_idx, "mol_slot": mol_slot,
        "Wm1r": np.asarray(inp["Wm1r"], np.float32),
        "Wm1s": np.asarray(inp["Wm1s"], np.float32),
        "Wm2r": np.asarray(inp["Wm2r"], np.float32),
        "Wm2s": np.asarray(inp["Wm2s"], np.float32),
        "bm1_rep": _rep(inp["bm1"]), "bm2_rep": _rep(inp["bm2"]),
        "Wlm": np.asarray(inp["Wlm"], np.float32), "blm_col": _col(inp["blm"]),
        "Wlb": np.asarray(inp["Wlb"], np.float32), "blb_col": _col(inp["blb"]),
        "Wd1": np.asarray(inp["Wd1"], np.float32),
        "bd1_t": _btile(inp["bd1"], 125, 4),
        "Wd2": np.asarray(inp["Wd2"], np.float32),
        "bd2_t": _btile(inp["bd2"], 128, 2),
        "Wcat1": np.asarray(inp["Wcat1"], np.float16),
        "bcat1_t": _btile(inp["bcat1"], 125, 8),
        "Wcat2": np.asarray(inp["Wcat2"], np.float32),
        "bcat2_t": np.asarray(inp["bcat2"], np.float32).reshape(1, 1),
        "bl1c_t": _btile(inp["bl1c"], 125, 8),
        "Wl2c": np.asarray(inp["Wl2c"], np.float16),
        "bl2c_t": _btile(inp["bl2c"], 125, 8),
        "Wl3c": np.asarray(inp["Wl3c"], np.float16),
        "bl3c_t": _btile(inp["bl3c"], 128, 2),
        "iota32": iota, "ident32": ident,
        "ones32": np.ones((P, 1), np.float32),
    }
    in_maps = []
    for c in range(NCORES):
        m = dict(shared)
        lo = c * CLL_NPC
        # xcllT packed [128, 4*512]: [p, k*512+n] = x_cll.T[k*128+p, lo+n]
        m["xcllT"] = np.ascontiguousarray(
            xcT[:, lo:lo + CLL_NPC].reshape(4, P, CLL_NPC)
            .transpose(1, 0, 2).reshape(P, 4 * CLL_NPC)).astype(np.float16)
        # Qt packed [128, 27*4*128]: [p, (s*4+b)*128+d] = q[s*128+p, lo+b*128+d]
        qc = q[:CLL_NCH * P, lo:lo + CLL_NPC]
        m["Qt"] = np.ascontiguousarray(
            qc.reshape(CLL_NCH, P, CLL_NBLK, P).transpose(1, 0, 2, 3)
            .reshape(P, CLL_NCH * CLL_NBLK * P)).astype(np.float16)
        # W1ct [128, 12*1000]: [p, (ch*4+blk)*1000+q] = w1c_n[lo+blk*128+p, ch, q]
        wslice = w1c_n[lo:lo + CLL_NPC]                         # [512, 3, 1000]
        m["W1ct"] = np.ascontiguousarray(
            wslice.reshape(CLL_NBLK, P, 3, 1000).transpose(1, 2, 0, 3)
            .reshape(P, 12 * 1000)).astype(np.float16)
        in_maps.append(m)
    return in_maps, meta


# ------------------------------------------------------------ device program

RG = [list(range(NCORES))]


def _declare_inputs(nc):
    spec = {
        "xcllT": ([P, 4 * CLL_NPC], f16),
        "Qt": ([P, CLL_NCH * CLL_NBLK * P], f16),
        "Wc1": ([P, 4 * F], f16), "Wc2": ([F, F], f16), "Wc3": ([F, F], f16),
        "Wc4": ([F, 3], f16),
        "bc1_rep": ([P, F], f32), "bc2_rep": ([P, F], f32),
        "bc3_rep": ([P, F], f32), "bc4_rep": ([P, 3], f32),
        "W1ct": ([P, 12 * 1000], f16),
        "xbioT": ([256, BIO_S2], f16),
        "Qb1": ([P, BIO_NCH * BIO_S1], f16), "Qb2": ([BIO_S1, 1], f16),
        "Wb1": ([256, F], f16), "Wb2": ([F, F], f16),
        "bb1_rep": ([P, F], f32), "bb2_row": ([1, F], f32),
        "x_mol": ([MOL_N, 64], f32), "xmolT": ([64, MOL_N], f32),
        "mol_idx": ([P, 8], i16), "mol_slot": ([P, 1], f32),
        "Wm1r": ([64, F], f32), "Wm1s": ([64, F], f32),
        "Wm2r": ([F, F], f32), "Wm2s": ([F, F], f32),
        "bm1_rep": ([P, F], f32), "bm2_rep": ([P, F], f32),
        "Wlm": ([F, 128], f32), "blm_col": ([128, 1], f32),
        "Wlb": ([F, 128], f32), "blb_col": ([128, 1], f32),
        "Wd1": ([256, 500], f32), "bd1_t": ([125, 4], f32),
        "Wd2": ([500, 256], f32), "bd2_t": ([128, 2], f32),
        "Wcat1": ([512, 1000], f16), "bcat1_t": ([125, 8], f32),
        "Wcat2": ([1000, 1], f32), "bcat2_t": ([1, 1], f32),
        "bl1c_t": ([125, 8], f32),
        "Wl2c": ([1000, 1000], f16), "bl2c_t": ([125, 8], f32),
        "Wl3c": ([1000, 256], f16), "bl3c_t": ([128, 2], f32),
        "iota32": ([P, P], f32), "ident32": ([P, P], f32),
        "ones32": ([P, 1], f32),
    }
    return {k: nc.dram_tensor(k, s, d, kind="ExternalInput")
            for k, (s, d) in spec.items()}


def build_program(meta=None, repeat=1):
    nc = bacc.Bacc("TRN2", target_bir_lowering=False, debug=False,
                   enable_asserts=False, num_devices=NCORES,
                   num_swdge_queues=4)
    io = _declare_inputs(nc)
    out = nc.dram_tensor("out", [1, 1], f32, kind="ExternalOutput")

    h_slice = [nc.dram_tensor(f"h{l}_slice", [CLL_NPC, F], f16,
                              kind="Internal") for l in range(4)]
    h_full = [nc.dram_tensor(f"h{l}_full", [CLL_PAD, F], f16,
                             kind="Internal", addr_space="Shared")
              for l in range(4)]
    m1_dram = nc.dram_tensor("m1_dram", [MOL_N, 256], f32, kind="Internal")
    ar_in = nc.dram_tensor("ar_in", [1000], f32, kind="Internal")
    ar_out = nc.dram_tensor("ar_out", [1000], f32, kind="Internal",
                            addr_space="Shared")

    with tile.TileContext(nc) as tc:
        for _ in range(repeat):
            _build(nc, tc, io, out, h_slice, h_full, m1_dram, ar_in, ar_out)
    nc.compile()
    return nc


def _build(nc, tc, io, out, h_slice, h_full, m1_dram, ar_in, ar_out):
    with (
        tc.tile_pool(name="const", bufs=1) as cp,
        tc.tile_pool(name="wp", bufs=1) as wp,
        tc.tile_pool(name="hp", bufs=2) as hp,
        tc.tile_pool(name="sb", bufs=3) as sb,
        tc.tile_pool(name="ct", bufs=2) as ctp,
        tc.tile_pool(name="psA", bufs=1, space="PSUM") as psA,
        tc.tile_pool(name="psT", bufs=2, space="PSUM") as psT,
        tc.tile_pool(name="psM", bufs=2, space="PSUM") as psM,
    ):
        def load(pool, name, rows=None, cols=None, tag=None, dt=None,
                 eng=None):
            src = io[name]
            r = rows if rows is not None else src.shape[0]
            c = cols if cols is not None else src.shape[1]
            t = pool.tile([r, c], dt or src.dtype, tag=tag or name)
            (eng or nc.sync).dma_start(t[:], src[0:r, 0:c])
            return t

        def load2(name, tag, rows=F, cols=F, eng=None):
            """[rows>128, cols] -> two tiles [128, cols] + [rows-128, cols]."""
            a = load(wp, name, rows=P, cols=cols, tag=tag + "a", eng=eng)
            b = wp.tile([P, cols], io[name].dtype, tag=tag + "b")
            (eng or nc.sync).dma_start(b[0:rows - P, :], io[name][P:rows, 0:cols])
            return a, b

        iota32 = load(cp, "iota32")
        ident32 = load(cp, "ident32")
        ones32 = load(cp, "ones32")

        def loadrows(name, nparts, cols, tag, rows=P, eng=None):
            """Tall [nparts*rows?, cols] tensor -> list of [128, cols] tiles."""
            ts = []
            for k in range(nparts):
                t = wp.tile([rows, cols], io[name].dtype, tag=f"{tag}{k}")
                (eng or nc.sync).dma_start(
                    t[:], io[name][k * rows:(k + 1) * rows, 0:cols])
                ts.append(t)
            return ts

        # ---- phase A: h1 = x_cll @ Wc1 (own slice), then AG1 ----
        xc = load(wp, "xcllT")                     # [128, 4*512] f16, k-major
        wc1 = load(wp, "Wc1")                      # [128, 4*200] f16, k-major
        for b in range(CLL_NBLK):
            ps = psM.tile([P, F], f32, tag="m", space="PSUM")
            for k in range(4):
                nc.tensor.matmul(ps[:],
                                 xc[:, k * CLL_NPC + b * P:
                                    k * CLL_NPC + (b + 1) * P],
                                 wc1[:, k * F:(k + 1) * F],
                                 start=(k == 0), stop=(k == 3))
            hst = sb.tile([P, F], f16, tag="hst")
            nc.vector.tensor_copy(hst[:], ps[:])
            nc.scalar.dma_start(h_slice[0][b * P:(b + 1) * P, 0:F], hst[:])

        def allgather(l):
            nc.gpsimd.collective_compute(
                "AllGather", mybir.AluOpType.bypass, replica_groups=RG,
                ins=[h_slice[l].ap()], outs=[h_full[l].ap()])

        allgather(0)

        # ---- weight/const loads that can stream during AG1 ----
        qt = load(wp, "Qt", eng=nc.scalar)         # [128, 13824] f16
        wc2 = load2("Wc2", "wc2", eng=nc.scalar)
        wc3 = load2("Wc3", "wc3", eng=nc.scalar)
        wc4 = load2("Wc4", "wc4", cols=3, eng=nc.scalar)
        bc_rep = [load(wp, f"bc{l}_rep", eng=nc.scalar) for l in (1, 2, 3)]
        bc4r = load(wp, "bc4_rep", eng=nc.scalar)

        PIECES = [(0, 7), (7, 14), (14, 21), (21, CLL_NCH)]

        def hload(l):
            ts = []
            src = h_full[l].ap().rearrange("(c p) f -> p c f", p=P)
            for pi, (s0, s1) in enumerate(PIECES):
                t = hp.tile([P, s1 - s0, F], f16, tag=f"hft{pi}",
                            name=f"hft{pi}")
                nc.sync.dma_start(t[:], src[:, s0:s1, :])
                ts.append(t)
            return ts

        def transpose_to(src_sb, dst0, dst1, bcol):
            """src [128, 200] f32 -> dst0[128, bcol:+128], dst1[72, bcol:+128] f16."""
            pt = psT.tile([P, P], f32, tag="tp", space="PSUM")
            nc.tensor.transpose(pt[0:P, 0:P], src_sb[:, 0:P], ident32[:])
            nc.vector.tensor_copy(dst0[:, bcol:bcol + P], pt[0:P, 0:P])
            pt2 = psT.tile([P, P], f32, tag="tp", space="PSUM")
            nc.tensor.transpose(pt2[0:F - P, 0:P], src_sb[:, P:F], ident32[:])
            nc.vector.tensor_copy(dst1[0:F - P, bcol:bcol + P],
                                  pt2[0:F - P, 0:P])

        def agg_blocks(hfts):
            """Piecewise-pipelined aggregation: 4 PSUM accumulators, matmuls
            grouped by table piece so compute starts on the first piece."""
            pss = [psA.tile([P, F], f32, tag=f"agg{b}", name=f"agg{b}",
                            space="PSUM") for b in range(CLL_NBLK)]
            for pi, (s0, s1) in enumerate(PIECES):
                for b in range(CLL_NBLK):
                    for s in range(s0, s1):
                        qcol = (s * CLL_NBLK + b) * P
                        nc.tensor.matmul(pss[b][:], qt[:, qcol:qcol + P],
                                         hfts[pi][:, s - s0, 0:F],
                                         start=(s == 0),
                                         stop=(s == CLL_NCH - 1))
            return pss

        def cll_layer(l, hfts, wnext, brep):
            """Aggregate layer l from table pieces; produce h_slice[l+1]."""
            cT0 = ctp.tile([P, CLL_NPC], f16, tag="cT0")
            cT1 = ctp.tile([P, CLL_NPC], f16, tag="cT1")
            pss = agg_blocks(hfts)
            for b in range(CLL_NBLK):
                t2 = sb.tile([P, F], f32, tag="ev1")
                nc.vector.tensor_tensor(t2[:], pss[b][:], brep[:], op=ADD)
                cblk = sb.tile([P, F], f32, tag="cblk", bufs=4)
                nc.scalar.activation(cblk[:], t2[:], RELU)
                if wnext is not None:
                    transpose_to(cblk, cT0, cT1, b * P)
                    wa, wb_ = wnext
                    ph = psM.tile([P, F], f32, tag="m", space="PSUM")
                    nc.tensor.matmul(ph[:], cT0[:, b * P:(b + 1) * P], wa[:],
                                     start=True, stop=False)
                    nc.tensor.matmul(ph[:], cT1[0:F - P, b * P:(b + 1) * P],
                                     wb_[0:F - P, :], start=False, stop=True)
                    hst = sb.tile([P, F], f16, tag="hst")
                    nc.vector.tensor_copy(hst[:], ph[:])
                    nc.scalar.dma_start(h_slice[l + 1][b * P:(b + 1) * P,
                                                       0:F], hst[:])
                else:
                    hst = sb.tile([P, F], f16, tag="hst")
                    nc.vector.tensor_copy(hst[:], cblk[:])
                    nc.sync.dma_start(h_slice[l + 1][b * P:(b + 1) * P, 0:F],
                                      hst[:])

        # ---- layer 1 ----
        cll_layer(0, hload(0), wc2, bc_rep[0])
        allgather(1)

        # ---- bio mini-branch (runs while AG2 is in flight) ----
        xbT = loadrows("xbioT", 2, BIO_S2, "xbT", eng=nc.sync)
        qb1 = load(wp, "Qb1", eng=nc.sync)
        qb2 = load(wp, "Qb2", eng=nc.sync)
        wb1 = loadrows("Wb1", 2, F, "wb1", eng=nc.sync)
        wb2 = load2("Wb2", "wb2", eng=nc.sync)
        bb1r = load(wp, "bb1_rep", eng=nc.sync)
        bb2row = load(wp, "bb2_row", eng=nc.sync)
        h1b = sb.tile([P, BIO_NCH, F], f16, tag="h1b", bufs=1)
        for j in range(BIO_NCH):
            ps = psM.tile([P, F], f32, tag="m", space="PSUM")
            for k in range(2):
                nc.tensor.matmul(ps[:], xbT[k][:, j * P:(j + 1) * P],
                                 wb1[k][:],
                                 start=(k == 0), stop=(k == 1))
            nc.vector.tensor_copy(h1b[:, j, :], ps[:])
        psb = psM.tile([P, F], f32, tag="m", space="PSUM")
        for j in range(BIO_NCH):
            nc.tensor.matmul(psb[:], qb1[:, j * P:(j + 1) * P], h1b[:, j, 0:F],
                             start=(j == 0), stop=(j == BIO_NCH - 1))
        tb1 = sb.tile([P, F], f32, tag="ev1")
        nc.vector.tensor_tensor(tb1[:], psb[:], bb1r[:], op=ADD)
        c1b = sb.tile([P, F], f32, tag="c1b", bufs=1)
        nc.scalar.activation(c1b[:], tb1[:], RELU)
        c1bT0 = sb.tile([P, P], f16, tag="c1bT0", bufs=1)
        c1bT1 = sb.tile([P, P], f16, tag="c1bT1", bufs=1)
        ptb = psT.tile([P, P], f32, tag="tp", space="PSUM")
        nc.tensor.transpose(ptb[0:P, 0:P], c1b[:, 0:P], ident32[:])
        nc.vector.tensor_copy(c1bT0[:], ptb[0:P, 0:P])
        ptb2 = psT.tile([P, P], f32, tag="tp", space="PSUM")
        nc.tensor.transpose(ptb2[0:F - P, 0:P], c1b[:, P:F], ident32[:])
        nc.vector.tensor_copy(c1bT1[0:F - P, :], ptb2[0:F - P, 0:P])
        ph2 = psM.tile([P, F], f32, tag="m", space="PSUM")
        nc.tensor.matmul(ph2[:], c1bT0[:, 0:P], wb2[0][:],
                         start=True, stop=False)
        nc.tensor.matmul(ph2[:], c1bT1[0:F - P, 0:P], wb2[1][0:F - P, :],
                         start=False, stop=True)
        h2b = sb.tile([P, F], f16, tag="h2b", bufs=1)
        nc.vector.tensor_copy(h2b[:], ph2[:])
        pr = psM.tile([1, F], f32, tag="m", space="PSUM")
        nc.tensor.matmul(pr[:], qb2[:], h2b[:], start=True, stop=True)
        tb2 = sb.tile([1, F], f32, tag="ev1")
        nc.vector.tensor_tensor(tb2[0:1, :], pr[0:1, :], bb2row[0:1, :], op=ADD)
        brow = sb.tile([1, F], f32, tag="brow", bufs=1)
        nc.scalar.activation(brow[0:1, :], tb2[0:1, :], RELU)
        # bvec column [200, 1] for the head
        bgc0 = sb.tile([P, 1], f32, tag="bgc0", bufs=1)
        bgc1 = sb.tile([P, 1], f32, tag="bgc1", bufs=1)
        prc = psT.tile([P, P], f32, tag="tp", space="PSUM")
        nc.tensor.transpose(prc[0:P, 0:1], brow[0:1, 0:P], ident32[0:1, 0:1])
        nc.vector.tensor_copy(bgc0[:], prc[0:P, 0:1])
        prc2 = psT.tile([P, P], f32, tag="tp", space="PSUM")
        nc.tensor.transpose(prc2[0:F - P, 0:1], brow[0:1, P:F],
                            ident32[0:1, 0:1])
        nc.vector.tensor_copy(bgc1[0:F - P, :], prc2[0:F - P, 0:1])

        # ---- layer 2 ----
        cll_layer(1, hload(1), wc3, bc_rep[1])
        allgather(2)

        # ---- mol branch (runs while AG3 is in flight) ----
        mol_idx_sb = load(cp, "mol_idx")
        mol_slot_sb = load(cp, "mol_slot")
        xmolT_sb = load(wp, "xmolT", eng=nc.sync)
        wm1r = load(wp, "Wm1r", eng=nc.sync)
        wm1s = load(wp, "Wm1s", eng=nc.sync)
        bm1r = load(wp, "bm1_rep", eng=nc.sync)
        bm2r = load(wp, "bm2_rep", eng=nc.sync)
        v1 = sb.tile([P, 1, 64], f32, tag="vm")
        nc.gpsimd.dma_gather(v1[:], io["x_mol"].ap(), mol_idx_sb[:],
                             MOL_E, MOL_E, 64)
        mM = sb.tile([P, 64], f32, tag="Mmol", bufs=1)
        nc.vector.tensor_scalar(mM[:], iota32[:, 0:64], mol_slot_sb[:, 0:1],
                                None, op0=EQ)
        agg_ps = psM.tile([64, 64], f32, tag="m", space="PSUM")
        nc.tensor.matmul(agg_ps[:], mM[:], v1[:, 0, :], start=True, stop=True)
        agg_sb = sb.tile([64, 64], f32, tag="mol1")
        nc.vector.tensor_copy(agg_sb[:], agg_ps[:])
        pt = psT.tile([P, P], f32, tag="tp", space="PSUM")
        nc.tensor.transpose(pt[0:64, 0:64], agg_sb[0:64, 0:64],
                            ident32[0:64, 0:64])
        aggT = sb.tile([64, 64], f32, tag="mol2")
        nc.vector.tensor_copy(aggT[:], pt[0:64, 0:64])
        h1_ps = psM.tile([64, F], f32, tag="m", space="PSUM")
        nc.tensor.matmul(h1_ps[:], aggT[:], wm1r[:], start=True, stop=False)
        nc.tensor.matmul(h1_ps[:], xmolT_sb[:], wm1s[:], start=False, stop=True)
        t_m1 = sb.tile([64, F], f32, tag="mol3")
        nc.vector.tensor_tensor(t_m1[:], h1_ps[:], bm1r[0:64, :], op=ADD)
        m1_sb = sb.tile([64, F], f32, tag="mol4", bufs=1)
        nc.scalar.activation(m1_sb[:], t_m1[:], RELU)
        nc.scalar.dma_start(m1_dram[0:64, 0:F], m1_sb[:])

        wm2r = load2("Wm2r", "wm2r", eng=nc.sync)
        wm2s = load2("Wm2s", "wm2s", eng=nc.sync)
        v2 = sb.tile([P, 1, 256], f32, tag="vm2")
        nc.gpsimd.dma_gather(v2[:], m1_dram.ap(), mol_idx_sb[:],
                             MOL_E, MOL_E, 256)
        agg2_ps = psM.tile([64, F], f32, tag="m", space="PSUM")
        nc.tensor.matmul(agg2_ps[:], mM[:], v2[:, 0, 0:F], start=True, stop=True)
        agg2_sb = sb.tile([64, F], f32, tag="mol1")
        nc.vector.tensor_copy(agg2_sb[:], agg2_ps[:])
        a2T0 = sb.tile([P, 64], f32, tag="mol5")
        a2T1 = sb.tile([P, 64], f32, tag="mol6")
        m1T0 = sb.tile([P, 64], f32, tag="mol7")
        m1T1 = sb.tile([P, 64], f32, tag="mol8")
        for srcT, d0, d1 in ((agg2_sb, a2T0, a2T1), (m1_sb, m1T0, m1T1)):
            pt1 = psT.tile([P, P], f32, tag="tp", space="PSUM")
            nc.tensor.transpose(pt1[0:P, 0:64], srcT[0:64, 0:P],
                                ident32[0:64, 0:64])
            nc.vector.tensor_copy(d0[:, 0:64], pt1[0:P, 0:64])
            pt2 = psT.tile([P, P], f32, tag="tp", space="PSUM")
            nc.tensor.transpose(pt2[0:F - P, 0:64], srcT[0:64, P:F],
                                ident32[0:64, 0:64])
            nc.vector.tensor_copy(d1[0:F - P, 0:64], pt2[0:F - P, 0:64])
        h2_ps = psM.tile([64, F], f32, tag="m", space="PSUM")
        nc.tensor.matmul(h2_ps[:], a2T0[:, 0:64], wm2r[0][:],
                         start=True, stop=False)
        nc.tensor.matmul(h2_ps[:], a2T1[0:F - P, 0:64], wm2r[1][0:F - P, :],
                         start=False, stop=False)
        nc.tensor.matmul(h2_ps[:], m1T0[:, 0:64], wm2s[0][:],
                         start=False, stop=False)
        nc.tensor.matmul(h2_ps[:], m1T1[0:F - P, 0:64], wm2s[1][0:F - P, :],
                         start=False, stop=True)
        t_m2 = sb.tile([64, F], f32, tag="mol3")
        nc.vector.tensor_tensor(t_m2[:], h2_ps[:], bm2r[0:64, :], op=ADD)
        m2_sb = sb.tile([64, F], f32, tag="mol4", bufs=1)
        nc.scalar.activation(m2_sb[:], t_m2[:], RELU)

        wlm = load2("Wlm", "wlm", cols=128, eng=nc.sync)
        blm = load(wp, "blm_col", eng=nc.sync)
        mcol0 = sb.tile([P, 1], f32, tag="mc0", bufs=1)
        mcol1 = sb.tile([P, 1], f32, tag="mc1", bufs=1)
        pool_ps = psM.tile([P, 1], f32, tag="m", space="PSUM")
        nc.tensor.matmul(pool_ps[0:P, :], m2_sb[0:64, 0:P], ones32[0:64, :],
                         start=True, stop=True)
        nc.scalar.activation(mcol0[:], pool_ps[0:P, :], COPY, scale=1.0 / 64.0)
        pool_ps2 = psM.tile([P, 1], f32, tag="m", space="PSUM")
        nc.tensor.matmul(pool_ps2[0:F - P, :], m2_sb[0:64, P:F],
                         ones32[0:64, :], start=True, stop=True)
        nc.scalar.activation(mcol1[0:F - P, :], pool_ps2[0:F - P, :], COPY,
                             scale=1.0 / 64.0)
        mvec = sb.tile([P, 1], f32, tag="mvec", bufs=1)
        mm_ps = psM.tile([P, 1], f32, tag="m", space="PSUM")
        nc.tensor.matmul(mm_ps[:], wlm[0][:], mcol0[:], start=True, stop=False)
        nc.tensor.matmul(mm_ps[:], wlm[1][0:F - P, :], mcol1[0:F - P, :],
                         start=False, stop=True)
        nc.scalar.activation(mvec[:], mm_ps[:], RELU, bias=blm[:])

        # ---- layer 3 (produces c3 slices -> AG4) ----
        cll_layer(2, hload(2), None, bc_rep[2])
        allgather(3)

        # ---- head weight loads (stream during AG4) ----
        wlb = load2("Wlb", "wlb", cols=128, eng=nc.sync)
        blb = load(wp, "blb_col", eng=nc.sync)
        wd1 = load2("Wd1", "wd1", rows=256, cols=500, eng=nc.sync)
        bd1 = load(wp, "bd1_t", eng=nc.sync)
        wd2t = [wp.tile([125, 256], f32, tag=f"wd2_{k}", name=f"wd2_{k}")
                for k in range(4)]
        for k in range(4):
            nc.sync.dma_start(wd2t[k][:], io["Wd2"][k * 125:(k + 1) * 125, :])
        bd2 = load(wp, "bd2_t", eng=nc.sync)
        bl1c = load(wp, "bl1c_t", eng=nc.sync)
        bl2c = load(wp, "bl2c_t", eng=nc.sync)
        wtc2 = [wp.tile([125, 1000], f16, tag=f"wl2c_{k}", name=f"wl2c_{k}")
                for k in range(8)]
        for k in range(8):
            nc.sync.dma_start(wtc2[k][:], io["Wl2c"][k * 125:(k + 1) * 125, :])
        bl3c = load(wp, "bl3c_t", eng=nc.sync)
        wtc3 = [wp.tile([125, 256], f16, tag=f"wl3c_{k}", name=f"wl3c_{k}")
                for k in range(8)]
        for k in range(8):
            nc.sync.dma_start(wtc3[k][:], io["Wl3c"][k * 125:(k + 1) * 125, :])
        bcat1 = load(wp, "bcat1_t", eng=nc.sync)
        wtu = [wp.tile([P, 1000], f16, tag=f"wcat1_{k}", name=f"wcat1_{k}")
               for k in range(4)]
        for k in range(4):
            nc.sync.dma_start(wtu[k][:], io["Wcat1"][k * P:(k + 1) * P, :])
        wcat2 = wp.tile([125, 8], f32, tag="wcat2")
        for k in range(8):
            nc.sync.dma_start(wcat2[:, k:k + 1],
                              io["Wcat2"][k * 125:(k + 1) * 125, 0:1])
        bcat2 = load(wp, "bcat2_t", eng=nc.sync)
        w1ct = load(wp, "W1ct", eng=nc.sync)       # [128, 12000] f16

        # ---- layer 4: aggregate c3, transform by Wc4, dense-1 partials ----
        pss4 = agg_blocks(hload(3))
        h4pack = sb.tile([P, 12], f16, tag="h4p", bufs=1)
        for b in range(CLL_NBLK):
            ag = sb.tile([P, F], f32, tag="ev1")
            nc.vector.tensor_copy(ag[:], pss4[b][:])
            aT0 = sb.tile([P, P], f16, tag="a4T0")
            aT1 = sb.tile([P, P], f16, tag="a4T1")
            pt4 = psT.tile([P, P], f32, tag="tp", space="PSUM")
            nc.tensor.transpose(pt4[0:P, 0:P], ag[:, 0:P], ident32[:])
            nc.vector.tensor_copy(aT0[:], pt4[0:P, 0:P])
            pt5 = psT.tile([P, P], f32, tag="tp", space="PSUM")
            nc.tensor.transpose(pt5[0:F - P, 0:P], ag[:, P:F], ident32[:])
            nc.vector.tensor_copy(aT1[0:F - P, :], pt5[0:F - P, 0:P])
            ph4 = psM.tile([P, 3], f32, tag="m", space="PSUM")
            nc.tensor.matmul(ph4[:], aT0[:, 0:P], wc4[0][:],
                             start=True, stop=False)
            nc.tensor.matmul(ph4[:], aT1[0:F - P, 0:P], wc4[1][0:F - P, :],
                             start=False, stop=True)
            th4 = sb.tile([P, 3], f32, tag="th4")
            nc.vector.tensor_tensor(th4[:], ph4[:], bc4r[:, 0:3], op=ADD)
            h4b = sb.tile([P, 3], f32, tag="h4b")
            nc.scalar.activation(h4b[:], th4[:], RELU)
            for ch in range(3):
                nc.vector.tensor_copy(h4pack[:, ch * 4 + b:ch * 4 + b + 1],
                                      h4b[:, ch:ch + 1])

        dsum = sb.tile([1, 1000], f32, tag="dsum", bufs=1)
        for half in range(2):
            psd = psM.tile([1, 500], f32, tag="m", space="PSUM")
            for j in range(12):
                nc.tensor.matmul(psd[:], h4pack[:, j:j + 1],
                                 w1ct[:, j * 1000 + half * 500:
                                      j * 1000 + half * 500 + 500],
                                 start=(j == 0), stop=(j == 11))
            nc.vector.tensor_copy(dsum[0:1, half * 500:half * 500 + 500],
                                  psd[0:1, :])
        nc.scalar.dma_start(ar_in.ap()[0:1000, None], dsum[0:1, :])

        nc.gpsimd.collective_compute(
            "AllReduce", mybir.AluOpType.add, replica_groups=RG,
            ins=[ar_in.ap()], outs=[ar_out.ap()])

        # ---- fusion head (replicated) ----
        def mm_chain(p_rows, n_cols, k_steps, act_bias, out_tag):
            acc = sb.tile([p_rows, n_cols], f32, tag=out_tag + "a")
            for k in range(k_steps):
                lhsT, rhs = yield k
                pst = psM.tile([p_rows, n_cols], f32, tag="m", space="PSUM")
                for och in range(n_cols):
                    nc.tensor.matmul(pst[:, och:och + 1], lhsT(och), rhs,
                                     start=True, stop=True)
                if k == 0:
                    nc.vector.tensor_copy(acc[:], pst[:])
                else:
                    nc.vector.tensor_tensor(acc[:], acc[:], pst[:], op=ADD)
            o = sb.tile([p_rows, n_cols], f32, tag=out_tag, bufs=1)
            for och in range(n_cols):
                nc.scalar.activation(o[:, och:och + 1], acc[:, och:och + 1],
                                     RELU, bias=act_bias[:, och:och + 1])
            yield o

        def run_chain(p_rows, n_cols, pieces, act_bias, out_tag):
            gen = mm_chain(p_rows, n_cols, len(pieces), act_bias, out_tag)
            k = next(gen)
            while True:
                r = gen.send(pieces[k])
                if not isinstance(r, int):
                    return r
                k = r

        bvec = run_chain(P, 1, [
            (lambda o: wlb[0][:, 0:128], bgc0[:]),
            (lambda o: wlb[1][0:F - P, 0:128], bgc1[0:F - P, :]),
        ], blb, "bvec")

        d1 = run_chain(125, 4, [
            (lambda o: wd1[0][:, o * 125:(o + 1) * 125], mvec[:]),
            (lambda o: wd1[1][:, o * 125:(o + 1) * 125], bvec[:]),
        ], bd1, "d1")

        d2 = run_chain(P, 2, [
            (lambda o, k=k: wd2t[k][:, o * P:(o + 1) * P], d1[:, k:k + 1])
            for k in range(4)
        ], bd2, "d2")

        c1 = sb.tile([125, 8], f32, tag="c1", bufs=1)
        for j in range(8):
            tmpc = sb.tile([125, 1], f32, tag="ctmp")
            nc.sync.dma_start(tmpc[:], ar_out.ap()[j * 125:(j + 1) * 125, None])
            nc.scalar.activation(c1[:, j:j + 1], tmpc[:], RELU,
                                 bias=bl1c[:, j:j + 1])
        c1h = sb.tile([125, 8], f16, tag="c1h", bufs=1)
        nc.vector.tensor_copy(c1h[:], c1[:])

        c2 = run_chain(125, 8, [
            (lambda o, k=k: wtc2[k][:, o * 125:(o + 1) * 125], c1h[:, k:k + 1])
            for k in range(8)
        ], bl2c, "c2")
        c2h = sb.tile([125, 8], f16, tag="c2h", bufs=1)
        nc.vector.tensor_copy(c2h[:], c2[:])

        c3 = run_chain(P, 2, [
            (lambda o, k=k: wtc3[k][:, o * P:(o + 1) * P], c2h[:, k:k + 1])
            for k in range(8)
        ], bl3c, "c3")

        cat_h = sb.tile([P, 4], f16, tag="cath", bufs=1)
        nc.vector.tensor_copy(cat_h[:, 0:1], d2[:, 0:1])
        nc.vector.tensor_copy(cat_h[:, 1:2], d2[:, 1:2])
        nc.vector.tensor_copy(cat_h[:, 2:3], c3[:, 0:1])
        nc.vector.tensor_copy(cat_h[:, 3:4], c3[:, 1:2])
        u = run_chain(125, 8, [
            (lambda o, k=k: wtu[k][:, o * 125:(o + 1) * 125], cat_h[:, k:k + 1])
            for k in range(4)
        ], bcat1, "u")

        pso = psM.tile([1, 1], f32, tag="m", space="PSUM")
        for k in range(8):
            nc.tensor.matmul(pso[:], wcat2[:, k:k + 1], u[:, k:k + 1],
                             start=(k == 0), stop=(k == 7))
        osb = sb.tile([1, 1], f32, tag="osb", bufs=1)
        nc.scalar.activation(osb[:], pso[:], RELU, bias=bcat2[:])
        nc.sync.dma_start(out[0:1, 0:1], osb[:])


# ------------------------------------------------------------------- entry

_CACHE = {}


def kernel(**inputs):
    in_maps, meta = prep_inputs(inputs)
    if "nc" not in _CACHE:
        _CACHE["nc"] = build_program(meta)
    nc = _CACHE["nc"]
    res = run_bass_kernel_spmd(nc, in_maps, core_ids=list(range(NCORES)))
    return np.asarray(res.results[0]["out"], np.float32)


# revision 23
# speedup vs baseline: 1.0644x; 1.0644x over previous
"""Trainium2 Bass kernel for nn_DrugRank (GNN message passing), 8 NeuronCores.

Architecture (v2 — dense-block aggregation, pruned bio branch):

  - The reference consumes only row -1 (node 49999) of the bio GCN stack, so
    the 800k-edge bio branch is dead code except the 2-hop in-neighborhood of
    that node: ~16 L2 edges and ~280 L1 edges. Host prep extracts that
    neighborhood; the device does a handful of matmuls, replicated per core.
  - cll graph (3451 nodes, 55216 edges, 4 GCN layers): dst-node sharded,
    512 nodes (4 blocks of 128) per core. All GCN normalization (symmetric
    deg^-1/2 + self loops) is folded host-side into dense per-(src-chunk,
    dst-block) adjacency tiles Q[s,d] = dinv_s*dinv_d*cnt(s->d) +
    dinv_d^2*[s==d]; aggregation = relu(Q^T h + b) via 27x4 PSUM-accumulated
    128x128x200 matmuls per layer. No dma_gather anywhere in the cll path;
    identical work on every core (no stragglers).
  - Per layer: transform own slice (h = c @ W via PE transposes), AllGather
    the 200KB slice, reload the 1.35MB table, aggregate. 4 AllGathers + one
    1000-float AllReduce (dense-1 partials) are the only collectives.
  - mol branch + fusion head replicated on every core; head weights stream
    during the cll chain. Dense-1 (10353x1000) row-sharded with the node
    sharding, reduced by the AllReduce.
"""

import numpy as np

import concourse.bacc as bacc
import concourse.bass as bass
import concourse.mybir as mybir
import concourse.tile as tile
from concourse.bass_utils import run_bass_kernel_spmd

NCORES = 8
P = 128
F = 200

CLL_N, CLL_E, CLL_PAD, CLL_NPC = 3451, 55216, 4096, 512
CLL_NBLK = CLL_NPC // P                 # 4 dst blocks / core
CLL_NCH = 27                            # src chunks with real nodes
N_BIO = 50000
BIO_S2, BIO_S1 = 768, 128               # padded bio 2-hop sets
BIO_NCH = BIO_S2 // P                   # 6
MOL_N, MOL_E = 64, 128

f32 = mybir.dt.float32
f16 = mybir.dt.float16
i16 = mybir.dt.int16
RELU = mybir.ActivationFunctionType.Relu
COPY = mybir.ActivationFunctionType.Copy
EQ = mybir.AluOpType.is_equal
MUL = mybir.AluOpType.mult
ADD = mybir.AluOpType.add


# ---------------------------------------------------------------- host prep

def _pack_idx16(flat):
    n = len(flat)
    a16 = np.asarray(flat, np.int16).reshape(n // 16, 16).T
    return np.ascontiguousarray(np.tile(a16, (8, 1)))


def _pack_slots(flat, dtype=np.float16):
    n = len(flat)
    return np.ascontiguousarray(
        np.asarray(flat, np.float64).astype(dtype).reshape(n // P, P).T)


def _col(v):
    return np.ascontiguousarray(np.asarray(v, np.float32).reshape(-1, 1))


def _rep(v, rows=P):
    return np.ascontiguousarray(
        np.tile(np.asarray(v, np.float32).reshape(1, -1), (rows, 1)))


def _btile(v, p, n):
    return np.ascontiguousarray(np.asarray(v, np.float32).reshape(n, p).T)


def _rowpad(v, n):
    """[m] -> [1, n] zero-padded row."""
    v = np.asarray(v, np.float32).reshape(-1)
    o = np.zeros((1, n), np.float32)
    o[0, :len(v)] = v
    return o


def _rowpack(w, rows_pad, cols, dt=np.float16):
    """[m, cols] -> [128, (rows_pad//128)*cols]: [p, j*cols+q] = w[j*128+p, q]."""
    w = np.asarray(w, np.float32)
    wp = np.zeros((rows_pad, cols), np.float32)
    wp[:w.shape[0]] = w
    nj = rows_pad // P
    return np.ascontiguousarray(
        wp.reshape(nj, P, cols).transpose(1, 0, 2).reshape(P, nj * cols)
    ).astype(dt)


def _cll_q(edge, dinv):
    """Dense normalized adjacency, [CLL_PAD, CLL_PAD] f32."""
    src = edge[0].astype(np.int64)
    dst = edge[1].astype(np.int64)
    q = np.zeros((CLL_PAD, CLL_PAD), np.float32)
    np.add.at(q, (src, dst), (dinv[src] * dinv[dst]).astype(np.float32))
    di = np.arange(CLL_N)
    q[di, di] += (dinv[:CLL_N] * dinv[:CLL_N]).astype(np.float32)
    return q


def _bio_prune(edge, x_bio):
    """2-hop in-neighborhood of node N_BIO-1 -> (xbT_sel, Qb1_pk, Qb2)."""
    src = edge[0].astype(np.int64)
    dst = edge[1].astype(np.int64)
    deg = np.bincount(dst, minlength=N_BIO).astype(np.float64) + 1.0
    dinv = 1.0 / np.sqrt(deg)
    tgt = N_BIO - 1

    m2 = dst == tgt
    s1 = np.unique(np.concatenate([src[m2], [tgt]]))
    assert len(s1) <= BIO_S1, len(s1)
    pos1 = np.full(N_BIO, -1, np.int64)
    pos1[s1] = np.arange(len(s1))

    m1 = pos1[dst] >= 0
    e1s, e1d = src[m1], dst[m1]
    s2 = np.unique(np.concatenate([e1s, s1]))
    assert len(s2) <= BIO_S2, len(s2)
    pos2 = np.full(N_BIO, -1, np.int64)
    pos2[s2] = np.arange(len(s2))

    q1 = np.zeros((BIO_S2, BIO_S1), np.float32)
    np.add.at(q1, (pos2[e1s], pos1[e1d]),
              (dinv[e1s] * dinv[e1d]).astype(np.float32))
    q1[pos2[s1], pos1[s1]] += (dinv[s1] * dinv[s1]).astype(np.float32)

    q2 = np.zeros((BIO_S1, 1), np.float32)
    np.add.at(q2, (pos1[src[m2]], 0),
              (dinv[src[m2]] * dinv[tgt]).astype(np.float32))
    q2[pos1[tgt], 0] += np.float32(dinv[tgt] * dinv[tgt])

    xsel = np.zeros((BIO_S2, 256), np.float32)
    xsel[:len(s2)] = x_bio[s2]
    xbT = np.ascontiguousarray(xsel.T).astype(np.float16)      # [256, 768]
    # Qb1 packed [128, 6*128]: [p, s*128+d] = q1[s*128+p, d]
    q1pk = np.ascontiguousarray(
        q1.reshape(BIO_NCH, P, BIO_S1).transpose(1, 0, 2)
        .reshape(P, BIO_NCH * BIO_S1)).astype(np.float16)
    return xbT, q1pk, q2.astype(np.float16)


def prep_inputs(inp):
    meta = {}
    # ---- cll Q tiles ----
    dst = inp["edge_cll"][1].astype(np.int64)
    deg = np.bincount(dst, minlength=CLL_N).astype(np.float64) + 1.0
    dinv = np.zeros(CLL_PAD, np.float64)
    dinv[:CLL_N] = 1.0 / np.sqrt(deg)
    q = _cll_q(inp["edge_cll"], dinv)

    xcT = np.zeros((512, CLL_PAD), np.float32)
    xcT[:, :CLL_N] = inp["x_cll"].T

    # W1c regrouped: rows (node*3+ch) -> per core [128, 12*1000] f16,
    # col-block j = ch*4+blk, rows = local node p of that block.
    w1c = np.asarray(inp["Wl1c"], np.float32)                  # [10353, 1000]
    w1c_n = np.zeros((CLL_PAD, 3, 1000), np.float32)
    w1c_n[:CLL_N] = w1c.reshape(CLL_N, 3, 1000)

    xbT_sel, q1pk, q2 = _bio_prune(inp["edge_bio"], np.asarray(inp["x_bio"]))

    mol_s = inp["edge_mol"][0].astype(np.int64)
    mol_d = inp["edge_mol"][1].astype(np.int64)
    order = np.argsort(mol_d, kind="stable")
    mol_idx = _pack_idx16(mol_s[order])
    mol_slot = _pack_slots(mol_d[order].astype(np.float64), np.float32)

    iota = np.tile(np.arange(P, dtype=np.float32), (P, 1))
    ident = np.eye(P, dtype=np.float32)

    wc1 = np.asarray(inp["Wc1"], np.float32)                   # [512, 200]
    shared = {
        "Wc1": np.ascontiguousarray(
            wc1.reshape(4, P, F).transpose(1, 0, 2)
            .reshape(P, 4 * F)).astype(np.float16),
        "Wc2": np.asarray(inp["Wc2"], np.float16),
        "Wc3": np.asarray(inp["Wc3"], np.float16),
        "Wc4": np.asarray(inp["Wc4"], np.float16),
        "bc1_rep": _rep(inp["bc1"]), "bc2_rep": _rep(inp["bc2"]),
        "bc3_rep": _rep(inp["bc3"]), "bc4_rep": _rep(inp["bc4"]),
        "xbioT": xbT_sel, "Qb1": q1pk, "Qb2": q2,
        "Wb1": np.asarray(inp["Wb1"], np.float16),
        "Wb2": np.asarray(inp["Wb2"], np.float16),
        "bb1_rep": _rep(inp["bb1"]),
        "bb2_row": np.ascontiguousarray(
            np.asarray(inp["bb2"], np.float32).reshape(1, -1)),
        "x_mol": np.asarray(inp["x_mol"], np.float32),
        "xmolT": np.ascontiguousarray(inp["x_mol"].T.astype(np.float32)),
        "mol_idx": mol_idx, "mol_slot": mol_slot,
        "Wm1r": np.asarray(inp["Wm1r"], np.float32),
        "Wm1s": np.asarray(inp["Wm1s"], np.float32),
        "Wm2r": np.asarray(inp["Wm2r"], np.float32),
        "Wm2s": np.asarray(inp["Wm2s"], np.float32),
        "bm1_rep": _rep(inp["bm1"]), "bm2_rep": _rep(inp["bm2"]),
        "Wlm": np.asarray(inp["Wlm"], np.float32), "blm_col": _col(inp["blm"]),
        "Wlb": np.asarray(inp["Wlb"], np.float32), "blb_col": _col(inp["blb"]),
        "Wd1": np.asarray(inp["Wd1"], np.float32),
        "bd1_t": _btile(inp["bd1"], 125, 4),
        "Wd2": np.asarray(inp["Wd2"], np.float32),
        "bd2_t": _btile(inp["bd2"], 128, 2),
        "Wcat1_pk": _rowpack(inp["Wcat1"], 512, 1000),
        "bcat1_row": _rowpad(inp["bcat1"], 1024),
        "Wcat2_pk": _rowpack(inp["Wcat2"], 1024, 1, np.float32),
        "bcat2_t": np.asarray(inp["bcat2"], np.float32).reshape(1, 1),
        "bl1c_pk": np.ascontiguousarray(
            _rowpad(inp["bl1c"], 1024).reshape(8, P).T),
        "Wl2c_pk": _rowpack(inp["Wl2c"], 1024, 1000),
        "bl2c_row": _rowpad(inp["bl2c"], 1024),
        "Wl3c_pk": _rowpack(inp["Wl3c"], 1024, 256),
        "bl3c_row": _rowpad(inp["bl3c"], 256),
        "iota32": iota, "ident32": ident, "ident16": ident.astype(np.float16),
        "ones32": np.ones((P, 1), np.float32),
    }
    in_maps = []
    for c in range(NCORES):
        m = dict(shared)
        lo = c * CLL_NPC
        # xcllT packed [128, 4*512]: [p, k*512+n] = x_cll.T[k*128+p, lo+n]
        m["xcllT"] = np.ascontiguousarray(
            xcT[:, lo:lo + CLL_NPC].reshape(4, P, CLL_NPC)
            .transpose(1, 0, 2).reshape(P, 4 * CLL_NPC)).astype(np.float16)
        # Qt packed [128, 27*4*128]: [p, (s*4+b)*128+d] = q[s*128+p, lo+b*128+d]
        qc = q[:CLL_NCH * P, lo:lo + CLL_NPC]
        m["Qt"] = np.ascontiguousarray(
            qc.reshape(CLL_NCH, P, CLL_NBLK, P).transpose(1, 0, 2, 3)
            .reshape(P, CLL_NCH * CLL_NBLK * P)).astype(np.float16)
        # W1ct [128, 12*1000]: [p, (ch*4+blk)*1000+q] = w1c_n[lo+blk*128+p, ch, q]
        wslice = w1c_n[lo:lo + CLL_NPC]                         # [512, 3, 1000]
        m["W1ct"] = np.ascontiguousarray(
            wslice.reshape(CLL_NBLK, P, 3, 1000).transpose(1, 2, 0, 3)
            .reshape(P, 12 * 1000)).astype(np.float16)
        in_maps.append(m)
    return in_maps, meta


# ------------------------------------------------------------ device program

RG = [list(range(NCORES))]


def _declare_inputs(nc):
    spec = {
        "xcllT": ([P, 4 * CLL_NPC], f16),
        "Qt": ([P, CLL_NCH * CLL_NBLK * P], f16),
        "Wc1": ([P, 4 * F], f16), "Wc2": ([F, F], f16), "Wc3": ([F, F], f16),
        "Wc4": ([F, 3], f16),
        "bc1_rep": ([P, F], f32), "bc2_rep": ([P, F], f32),
        "bc3_rep": ([P, F], f32), "bc4_rep": ([P, 3], f32),
        "W1ct": ([P, 12 * 1000], f16),
        "xbioT": ([256, BIO_S2], f16),
        "Qb1": ([P, BIO_NCH * BIO_S1], f16), "Qb2": ([BIO_S1, 1], f16),
        "Wb1": ([256, F], f16), "Wb2": ([F, F], f16),
        "bb1_rep": ([P, F], f32), "bb2_row": ([1, F], f32),
        "x_mol": ([MOL_N, 64], f32), "xmolT": ([64, MOL_N], f32),
        "mol_idx": ([P, 8], i16), "mol_slot": ([P, 1], f32),
        "Wm1r": ([64, F], f32), "Wm1s": ([64, F], f32),
        "Wm2r": ([F, F], f32), "Wm2s": ([F, F], f32),
        "bm1_rep": ([P, F], f32), "bm2_rep": ([P, F], f32),
        "Wlm": ([F, 128], f32), "blm_col": ([128, 1], f32),
        "Wlb": ([F, 128], f32), "blb_col": ([128, 1], f32),
        "Wd1": ([256, 500], f32), "bd1_t": ([125, 4], f32),
        "Wd2": ([500, 256], f32), "bd2_t": ([128, 2], f32),
        "Wcat1_pk": ([P, 4 * 1000], f16), "bcat1_row": ([1, 1024], f32),
        "Wcat2_pk": ([P, 8], f32), "bcat2_t": ([1, 1], f32),
        "bl1c_pk": ([P, 8], f32),
        "Wl2c_pk": ([P, 8 * 1000], f16), "bl2c_row": ([1, 1024], f32),
        "Wl3c_pk": ([P, 8 * 256], f16), "bl3c_row": ([1, 256], f32),
        "iota32": ([P, P], f32), "ident32": ([P, P], f32),
        "ident16": ([P, P], f16),
        "ones32": ([P, 1], f32),
    }
    return {k: nc.dram_tensor(k, s, d, kind="ExternalInput")
            for k, (s, d) in spec.items()}


def build_program(meta=None, repeat=1):
    nc = bacc.Bacc("TRN2", target_bir_lowering=False, debug=False,
                   enable_asserts=False, num_devices=NCORES,
                   num_swdge_queues=4)
    io = _declare_inputs(nc)
    out = nc.dram_tensor("out", [1, 1], f32, kind="ExternalOutput")

    h_slice = [nc.dram_tensor(f"h{l}_slice", [CLL_NPC, F], f16,
                              kind="Internal") for l in range(4)]
    h_full = [nc.dram_tensor(f"h{l}_full", [CLL_PAD, F], f16,
                             kind="Internal", addr_space="Shared")
              for l in range(4)]
    m1_dram = nc.dram_tensor("m1_dram", [MOL_N, 256], f32, kind="Internal")
    ar_in = nc.dram_tensor("ar_in", [1024], f32, kind="Internal")
    ar_out = nc.dram_tensor("ar_out", [1024], f32, kind="Internal",
                            addr_space="Shared")

    with tile.TileContext(nc) as tc:
        for _ in range(repeat):
            _build(nc, tc, io, out, h_slice, h_full, m1_dram, ar_in, ar_out)
    nc.compile()
    return nc


def _build(nc, tc, io, out, h_slice, h_full, m1_dram, ar_in, ar_out):
    with (
        tc.tile_pool(name="const", bufs=1) as cp,
        tc.tile_pool(name="wp", bufs=1) as wp,
        tc.tile_pool(name="hp", bufs=2) as hp,
        tc.tile_pool(name="sb", bufs=3) as sb,
        tc.tile_pool(name="ct", bufs=2) as ctp,
        tc.tile_pool(name="psA", bufs=1, space="PSUM") as psA,
        tc.tile_pool(name="psT", bufs=2, space="PSUM") as psT,
        tc.tile_pool(name="psM", bufs=2, space="PSUM") as psM,
    ):
        def load(pool, name, rows=None, cols=None, tag=None, dt=None,
                 eng=None):
            src = io[name]
            r = rows if rows is not None else src.shape[0]
            c = cols if cols is not None else src.shape[1]
            t = pool.tile([r, c], dt or src.dtype, tag=tag or name)
            (eng or nc.sync).dma_start(t[:], src[0:r, 0:c])
            return t

        def load2(name, tag, rows=F, cols=F, eng=None):
            """[rows>128, cols] -> two tiles [128, cols] + [rows-128, cols]."""
            a = load(wp, name, rows=P, cols=cols, tag=tag + "a", eng=eng)
            b = wp.tile([P, cols], io[name].dtype, tag=tag + "b")
            (eng or nc.sync).dma_start(b[0:rows - P, :], io[name][P:rows, 0:cols])
            return a, b

        iota32 = load(cp, "iota32")
        ident32 = load(cp, "ident32")
        ident16 = load(cp, "ident16")
        ones32 = load(cp, "ones32")

        def loadrows(name, nparts, cols, tag, rows=P, eng=None):
            """Tall [nparts*rows?, cols] tensor -> list of [128, cols] tiles."""
            ts = []
            for k in range(nparts):
                t = wp.tile([rows, cols], io[name].dtype, tag=f"{tag}{k}")
                (eng or nc.sync).dma_start(
                    t[:], io[name][k * rows:(k + 1) * rows, 0:cols])
                ts.append(t)
            return ts

        # ---- phase A: h1 = x_cll @ Wc1 (own slice), then AG1 ----
        xc = load(wp, "xcllT")                     # [128, 4*512] f16, k-major
        wc1 = load(wp, "Wc1")                      # [128, 4*200] f16, k-major
        for b in range(CLL_NBLK):
            ps = psM.tile([P, F], f32, tag="m", space="PSUM")
            for k in range(4):
                nc.tensor.matmul(ps[:],
                                 xc[:, k * CLL_NPC + b * P:
                                    k * CLL_NPC + (b + 1) * P],
                                 wc1[:, k * F:(k + 1) * F],
                                 start=(k == 0), stop=(k == 3))
            hst = sb.tile([P, F], f16, tag="hst")
            nc.vector.tensor_copy(hst[:], ps[:])
            nc.scalar.dma_start(h_slice[0][b * P:(b + 1) * P, 0:F], hst[:])

        def allgather(l):
            nc.gpsimd.collective_compute(
                "AllGather", mybir.AluOpType.bypass, replica_groups=RG,
                ins=[h_slice[l].ap()], outs=[h_full[l].ap()])

        allgather(0)

        # ---- weight/const loads that can stream during AG1 ----
        qt = wp.tile([P, CLL_NCH * CLL_NBLK * P], f16, tag="Qt")
        QHALF = 14 * CLL_NBLK * P
        nc.scalar.dma_start(qt[:, 0:QHALF], io["Qt"][:, 0:QHALF])
        nc.scalar.dma_start(qt[:, QHALF:], io["Qt"][:, QHALF:])
        wc2 = load2("Wc2", "wc2", eng=nc.scalar)
        wc3 = load2("Wc3", "wc3", eng=nc.scalar)
        wc4 = load2("Wc4", "wc4", cols=3, eng=nc.scalar)
        bc_rep = [load(wp, f"bc{l}_rep", eng=nc.scalar) for l in (1, 2, 3)]
        bc4r = load(wp, "bc4_rep", eng=nc.scalar)

        PIECES = [(0, 7), (7, 14), (14, 21), (21, CLL_NCH)]

        def hload(l):
            ts = []
            src = h_full[l].ap().rearrange("(c p) f -> p c f", p=P)
            for pi, (s0, s1) in enumerate(PIECES):
                t = hp.tile([P, s1 - s0, F], f16, tag=f"hft{pi}",
                            name=f"hft{pi}")
                nc.sync.dma_start(t[:], src[:, s0:s1, :])
                ts.append(t)
            return ts

        def transpose_to(src_sb, dst0, dst1, bcol):
            """src [128, 200] f32 -> dst0[128, bcol:+128], dst1[72, bcol:+128] f16."""
            pt = psT.tile([P, P], f32, tag="tp", space="PSUM")
            nc.tensor.transpose(pt[0:P, 0:P], src_sb[:, 0:P], ident32[:])
            nc.vector.tensor_copy(dst0[:, bcol:bcol + P], pt[0:P, 0:P])
            pt2 = psT.tile([P, P], f32, tag="tp", space="PSUM")
            nc.tensor.transpose(pt2[0:F - P, 0:P], src_sb[:, P:F], ident32[:])
            nc.vector.tensor_copy(dst1[0:F - P, bcol:bcol + P],
                                  pt2[0:F - P, 0:P])

        def agg_blocks(hfts):
            """Piecewise-pipelined aggregation: 4 PSUM accumulators, matmuls
            grouped by table piece so compute starts on the first piece."""
            pss = [psA.tile([P, F], f32, tag=f"agg{b}", name=f"agg{b}",
                            space="PSUM") for b in range(CLL_NBLK)]
            for pi, (s0, s1) in enumerate(PIECES):
                for b in range(CLL_NBLK):
                    for s in range(s0, s1):
                        qcol = (s * CLL_NBLK + b) * P
                        nc.tensor.matmul(pss[b][:], qt[:, qcol:qcol + P],
                                         hfts[pi][:, s - s0, 0:F],
                                         start=(s == 0),
                                         stop=(s == CLL_NCH - 1))
            return pss

        def cll_layer(l, hfts, wnext, brep):
            """Aggregate layer l from table pieces; produce h_slice[l+1]."""
            cT0 = ctp.tile([P, CLL_NPC], f16, tag="cT0")
            cT1 = ctp.tile([P, CLL_NPC], f16, tag="cT1")
            pss = agg_blocks(hfts)
            for b in range(CLL_NBLK):
                t2 = sb.tile([P, F], f32, tag="ev1")
                nc.vector.tensor_tensor(t2[:], pss[b][:], brep[:], op=ADD)
                cblk = sb.tile([P, F], f32, tag="cblk", bufs=4)
                nc.scalar.activation(cblk[:], t2[:], RELU)
                if wnext is not None:
                    transpose_to(cblk, cT0, cT1, b * P)
                    wa, wb_ = wnext
                    ph = psM.tile([P, F], f32, tag="m", space="PSUM")
                    nc.tensor.matmul(ph[:], cT0[:, b * P:(b + 1) * P], wa[:],
                                     start=True, stop=False)
                    nc.tensor.matmul(ph[:], cT1[0:F - P, b * P:(b + 1) * P],
                                     wb_[0:F - P, :], start=False, stop=True)
                    hst = sb.tile([P, F], f16, tag="hst")
                    nc.vector.tensor_copy(hst[:], ph[:])
                    nc.scalar.dma_start(h_slice[l + 1][b * P:(b + 1) * P,
                                                       0:F], hst[:])
                else:
                    hst = sb.tile([P, F], f16, tag="hst")
                    nc.vector.tensor_copy(hst[:], cblk[:])
                    nc.sync.dma_start(h_slice[l + 1][b * P:(b + 1) * P, 0:F],
                                      hst[:])

        # ---- layer 1 ----
        cll_layer(0, hload(0), wc2, bc_rep[0])
        allgather(1)

        # ---- bio mini-branch (runs while AG2 is in flight) ----
        xbT = loadrows("xbioT", 2, BIO_S2, "xbT", eng=nc.sync)
        qb1 = load(wp, "Qb1", eng=nc.sync)
        qb2 = load(wp, "Qb2", eng=nc.sync)
        wb1 = loadrows("Wb1", 2, F, "wb1", eng=nc.sync)
        wb2 = load2("Wb2", "wb2", eng=nc.sync)
        bb1r = load(wp, "bb1_rep", eng=nc.sync)
        bb2row = load(wp, "bb2_row", eng=nc.sync)
        h1b = sb.tile([P, BIO_NCH, F], f16, tag="h1b", bufs=1)
        for j in range(BIO_NCH):
            ps = psM.tile([P, F], f32, tag="m", space="PSUM")
            for k in range(2):
                nc.tensor.matmul(ps[:], xbT[k][:, j * P:(j + 1) * P],
                                 wb1[k][:],
                                 start=(k == 0), stop=(k == 1))
            nc.vector.tensor_copy(h1b[:, j, :], ps[:])
        psb = psM.tile([P, F], f32, tag="m", space="PSUM")
        for j in range(BIO_NCH):
            nc.tensor.matmul(psb[:], qb1[:, j * P:(j + 1) * P], h1b[:, j, 0:F],
                             start=(j == 0), stop=(j == BIO_NCH - 1))
        tb1 = sb.tile([P, F], f32, tag="ev1")
        nc.vector.tensor_tensor(tb1[:], psb[:], bb1r[:], op=ADD)
        c1b = sb.tile([P, F], f32, tag="c1b", bufs=1)
        nc.scalar.activation(c1b[:], tb1[:], RELU)
        c1bT0 = sb.tile([P, P], f16, tag="c1bT0", bufs=1)
        c1bT1 = sb.tile([P, P], f16, tag="c1bT1", bufs=1)
        ptb = psT.tile([P, P], f32, tag="tp", space="PSUM")
        nc.tensor.transpose(ptb[0:P, 0:P], c1b[:, 0:P], ident32[:])
        nc.vector.tensor_copy(c1bT0[:], ptb[0:P, 0:P])
        ptb2 = psT.tile([P, P], f32, tag="tp", space="PSUM")
        nc.tensor.transpose(ptb2[0:F - P, 0:P], c1b[:, P:F], ident32[:])
        nc.vector.tensor_copy(c1bT1[0:F - P, :], ptb2[0:F - P, 0:P])
        ph2 = psM.tile([P, F], f32, tag="m", space="PSUM")
        nc.tensor.matmul(ph2[:], c1bT0[:, 0:P], wb2[0][:],
                         start=True, stop=False)
        nc.tensor.matmul(ph2[:], c1bT1[0:F - P, 0:P], wb2[1][0:F - P, :],
                         start=False, stop=True)
        h2b = sb.tile([P, F], f16, tag="h2b", bufs=1)
        nc.vector.tensor_copy(h2b[:], ph2[:])
        pr = psM.tile([1, F], f32, tag="m", space="PSUM")
        nc.tensor.matmul(pr[:], qb2[:], h2b[:], start=True, stop=True)
        tb2 = sb.tile([1, F], f32, tag="ev1")
        nc.vector.tensor_tensor(tb2[0:1, :], pr[0:1, :], bb2row[0:1, :], op=ADD)
        brow = sb.tile([1, F], f32, tag="brow", bufs=1)
        nc.scalar.activation(brow[0:1, :], tb2[0:1, :], RELU)
        # bvec column [200, 1] for the head
        bgc0 = sb.tile([P, 1], f32, tag="bgc0", bufs=1)
        bgc1 = sb.tile([P, 1], f32, tag="bgc1", bufs=1)
        prc = psT.tile([P, P], f32, tag="tp", space="PSUM")
        nc.tensor.transpose(prc[0:P, 0:1], brow[0:1, 0:P], ident32[0:1, 0:1])
        nc.vector.tensor_copy(bgc0[:], prc[0:P, 0:1])
        prc2 = psT.tile([P, P], f32, tag="tp", space="PSUM")
        nc.tensor.transpose(prc2[0:F - P, 0:1], brow[0:1, P:F],
                            ident32[0:1, 0:1])
        nc.vector.tensor_copy(bgc1[0:F - P, :], prc2[0:F - P, 0:1])

        # ---- layer 2 ----
        cll_layer(1, hload(1), wc3, bc_rep[1])
        allgather(2)

        # ---- mol branch (runs while AG3 is in flight) ----
        mol_idx_sb = load(cp, "mol_idx")
        mol_slot_sb = load(cp, "mol_slot")
        xmolT_sb = load(wp, "xmolT", eng=nc.sync)
        wm1r = load(wp, "Wm1r", eng=nc.sync)
        wm1s = load(wp, "Wm1s", eng=nc.sync)
        bm1r = load(wp, "bm1_rep", eng=nc.sync)
        bm2r = load(wp, "bm2_rep", eng=nc.sync)
        v1 = sb.tile([P, 1, 64], f32, tag="vm")
        nc.gpsimd.dma_gather(v1[:], io["x_mol"].ap(), mol_idx_sb[:],
                             MOL_E, MOL_E, 64)
        mM = sb.tile([P, 64], f32, tag="Mmol", bufs=1)
        nc.vector.tensor_scalar(mM[:], iota32[:, 0:64], mol_slot_sb[:, 0:1],
                                None, op0=EQ)
        agg_ps = psM.tile([64, 64], f32, tag="m", space="PSUM")
        nc.tensor.matmul(agg_ps[:], mM[:], v1[:, 0, :], start=True, stop=True)
        agg_sb = sb.tile([64, 64], f32, tag="mol1")
        nc.vector.tensor_copy(agg_sb[:], agg_ps[:])
        pt = psT.tile([P, P], f32, tag="tp", space="PSUM")
        nc.tensor.transpose(pt[0:64, 0:64], agg_sb[0:64, 0:64],
                            ident32[0:64, 0:64])
        aggT = sb.tile([64, 64], f32, tag="mol2")
        nc.vector.tensor_copy(aggT[:], pt[0:64, 0:64])
        h1_ps = psM.tile([64, F], f32, tag="m", space="PSUM")
        nc.tensor.matmul(h1_ps[:], aggT[:], wm1r[:], start=True, stop=False)
        nc.tensor.matmul(h1_ps[:], xmolT_sb[:], wm1s[:], start=False, stop=True)
        t_m1 = sb.tile([64, F], f32, tag="mol3")
        nc.vector.tensor_tensor(t_m1[:], h1_ps[:], bm1r[0:64, :], op=ADD)
        m1_sb = sb.tile([64, F], f32, tag="mol4", bufs=1)
        nc.scalar.activation(m1_sb[:], t_m1[:], RELU)
        nc.scalar.dma_start(m1_dram[0:64, 0:F], m1_sb[:])

        wm2r = load2("Wm2r", "wm2r", eng=nc.sync)
        wm2s = load2("Wm2s", "wm2s", eng=nc.sync)
        v2 = sb.tile([P, 1, 256], f32, tag="vm2")
        nc.gpsimd.dma_gather(v2[:], m1_dram.ap(), mol_idx_sb[:],
                             MOL_E, MOL_E, 256)
        agg2_ps = psM.tile([64, F], f32, tag="m", space="PSUM")
        nc.tensor.matmul(agg2_ps[:], mM[:], v2[:, 0, 0:F], start=True, stop=True)
        agg2_sb = sb.tile([64, F], f32, tag="mol1")
        nc.vector.tensor_copy(agg2_sb[:], agg2_ps[:])
        a2T0 = sb.tile([P, 64], f32, tag="mol5")
        a2T1 = sb.tile([P, 64], f32, tag="mol6")
        m1T0 = sb.tile([P, 64], f32, tag="mol7")
        m1T1 = sb.tile([P, 64], f32, tag="mol8")
        for srcT, d0, d1 in ((agg2_sb, a2T0, a2T1), (m1_sb, m1T0, m1T1)):
            pt1 = psT.tile([P, P], f32, tag="tp", space="PSUM")
            nc.tensor.transpose(pt1[0:P, 0:64], srcT[0:64, 0:P],
                                ident32[0:64, 0:64])
            nc.vector.tensor_copy(d0[:, 0:64], pt1[0:P, 0:64])
            pt2 = psT.tile([P, P], f32, tag="tp", space="PSUM")
            nc.tensor.transpose(pt2[0:F - P, 0:64], srcT[0:64, P:F],
                                ident32[0:64, 0:64])
            nc.vector.tensor_copy(d1[0:F - P, 0:64], pt2[0:F - P, 0:64])
        h2_ps = psM.tile([64, F], f32, tag="m", space="PSUM")
        nc.tensor.matmul(h2_ps[:], a2T0[:, 0:64], wm2r[0][:],
                         start=True, stop=False)
        nc.tensor.matmul(h2_ps[:], a2T1[0:F - P, 0:64], wm2r[1][0:F - P, :],
                         start=False, stop=False)
        nc.tensor.matmul(h2_ps[:], m1T0[:, 0:64], wm2s[0][:],
                         start=False, stop=False)
        nc.tensor.matmul(h2_ps[:], m1T1[0:F - P, 0:64], wm2s[1][0:F - P, :],
                         start=False, stop=True)
        t_m2 = sb.tile([64, F], f32, tag="mol3")
        nc.vector.tensor_tensor(t_m2[:], h2_ps[:], bm2r[0:64, :], op=ADD)
        m2_sb = sb.tile([64, F], f32, tag="mol4", bufs=1)
        nc.scalar.activation(m2_sb[:], t_m2[:], RELU)

        wlm = load2("Wlm", "wlm", cols=128, eng=nc.sync)
        blm = load(wp, "blm_col", eng=nc.sync)
        mcol0 = sb.tile([P, 1], f32, tag="mc0", bufs=1)
        mcol1 = sb.tile([P, 1], f32, tag="mc1", bufs=1)
        pool_ps = psM.tile([P, 1], f32, tag="m", space="PSUM")
        nc.tensor.matmul(pool_ps[0:P, :], m2_sb[0:64, 0:P], ones32[0:64, :],
                         start=True, stop=True)
        nc.scalar.activation(mcol0[:], pool_ps[0:P, :], COPY, scale=1.0 / 64.0)
        pool_ps2 = psM.tile([P, 1], f32, tag="m", space="PSUM")
        nc.tensor.matmul(pool_ps2[0:F - P, :], m2_sb[0:64, P:F],
                         ones32[0:64, :], start=True, stop=True)
        nc.scalar.activation(mcol1[0:F - P, :], pool_ps2[0:F - P, :], COPY,
                             scale=1.0 / 64.0)
        mvec = sb.tile([P, 1], f32, tag="mvec", bufs=1)
        mm_ps = psM.tile([P, 1], f32, tag="m", space="PSUM")
        nc.tensor.matmul(mm_ps[:], wlm[0][:], mcol0[:], start=True, stop=False)
        nc.tensor.matmul(mm_ps[:], wlm[1][0:F - P, :], mcol1[0:F - P, :],
                         start=False, stop=True)
        nc.scalar.activation(mvec[:], mm_ps[:], RELU, bias=blm[:])

        # ---- layer 3 (produces c3 slices -> AG4) ----
        cll_layer(2, hload(2), None, bc_rep[2])
        allgather(3)

        # ---- head weight loads (stream during AG4) ----
        wlb = load2("Wlb", "wlb", cols=128, eng=nc.sync)
        blb = load(wp, "blb_col", eng=nc.sync)
        wd1 = load2("Wd1", "wd1", rows=256, cols=500, eng=nc.sync)
        bd1 = load(wp, "bd1_t", eng=nc.sync)
        wd2t = [wp.tile([125, 256], f32, tag=f"wd2_{k}", name=f"wd2_{k}")
                for k in range(4)]
        for k in range(4):
            nc.sync.dma_start(wd2t[k][:], io["Wd2"][k * 125:(k + 1) * 125, :])
        bd2 = load(wp, "bd2_t", eng=nc.sync)
        bl1c = load(wp, "bl1c_pk", eng=nc.sync)
        bl2c = load(wp, "bl2c_row", eng=nc.sync)
        wl2cpk = load(wp, "Wl2c_pk", eng=nc.sync)
        bl3c = load(wp, "bl3c_row", eng=nc.sync)
        wl3cpk = load(wp, "Wl3c_pk", eng=nc.sync)
        bcat1 = load(wp, "bcat1_row", eng=nc.sync)
        wcat1pk = load(wp, "Wcat1_pk", eng=nc.sync)
        wcat2pk = load(wp, "Wcat2_pk", eng=nc.sync)
        bcat2 = load(wp, "bcat2_t", eng=nc.sync)
        w1ct = load(wp, "W1ct", eng=nc.sync)       # [128, 12000] f16

        # ---- layer 4: aggregate c3, transform by Wc4, dense-1 partials ----
        pss4 = agg_blocks(hload(3))
        h4pack = sb.tile([P, 12], f16, tag="h4p", bufs=1)
        for b in range(CLL_NBLK):
            ag = sb.tile([P, F], f32, tag="ev1")
            nc.vector.tensor_copy(ag[:], pss4[b][:])
            aT0 = sb.tile([P, P], f16, tag="a4T0")
            aT1 = sb.tile([P, P], f16, tag="a4T1")
            pt4 = psT.tile([P, P], f32, tag="tp", space="PSUM")
            nc.tensor.transpose(pt4[0:P, 0:P], ag[:, 0:P], ident32[:])
            nc.vector.tensor_copy(aT0[:], pt4[0:P, 0:P])
            pt5 = psT.tile([P, P], f32, tag="tp", space="PSUM")
            nc.tensor.transpose(pt5[0:F - P, 0:P], ag[:, P:F], ident32[:])
            nc.vector.tensor_copy(aT1[0:F - P, :], pt5[0:F - P, 0:P])
            ph4 = psM.tile([P, 3], f32, tag="m", space="PSUM")
            nc.tensor.matmul(ph4[:], aT0[:, 0:P], wc4[0][:],
                             start=True, stop=False)
            nc.tensor.matmul(ph4[:], aT1[0:F - P, 0:P], wc4[1][0:F - P, :],
                             start=False, stop=True)
            th4 = sb.tile([P, 3], f32, tag="th4")
            nc.vector.tensor_tensor(th4[:], ph4[:], bc4r[:, 0:3], op=ADD)
            h4b = sb.tile([P, 3], f32, tag="h4b")
            nc.scalar.activation(h4b[:], th4[:], RELU)
            for ch in range(3):
                nc.vector.tensor_copy(h4pack[:, ch * 4 + b:ch * 4 + b + 1],
                                      h4b[:, ch:ch + 1])

        dsum = sb.tile([1, 1024], f32, tag="dsum", bufs=1)
        nc.vector.memset(dsum[0:1, 1000:1024], 0.0)
        for half in range(2):
            psd = psM.tile([1, 500], f32, tag="m", space="PSUM")
            for j in range(12):
                nc.tensor.matmul(psd[:], h4pack[:, j:j + 1],
                                 w1ct[:, j * 1000 + half * 500:
                                      j * 1000 + half * 500 + 500],
                                 start=(j == 0), stop=(j == 11))
            nc.vector.tensor_copy(dsum[0:1, half * 500:half * 500 + 500],
                                  psd[0:1, :])
        nc.scalar.dma_start(ar_in.ap()[0:1024, None], dsum[0:1, :])

        nc.gpsimd.collective_compute(
            "AllReduce", mybir.AluOpType.add, replica_groups=RG,
            ins=[ar_in.ap()], outs=[ar_out.ap()])

        # ---- fusion head (replicated) ----
        def mm_chain(p_rows, n_cols, k_steps, act_bias, out_tag):
            acc = sb.tile([p_rows, n_cols], f32, tag=out_tag + "a")
            for k in range(k_steps):
                lhsT, rhs = yield k
                pst = psM.tile([p_rows, n_cols], f32, tag="m", space="PSUM")
                for och in range(n_cols):
                    nc.tensor.matmul(pst[:, och:och + 1], lhsT(och), rhs,
                                     start=True, stop=True)
                if k == 0:
                    nc.vector.tensor_copy(acc[:], pst[:])
                else:
                    nc.vector.tensor_tensor(acc[:], acc[:], pst[:], op=ADD)
            o = sb.tile([p_rows, n_cols], f32, tag=out_tag, bufs=1)
            for och in range(n_cols):
                nc.scalar.activation(o[:, och:och + 1], acc[:, och:och + 1],
                                     RELU, bias=act_bias[:, och:och + 1])
            yield o

        def run_chain(p_rows, n_cols, pieces, act_bias, out_tag):
            gen = mm_chain(p_rows, n_cols, len(pieces), act_bias, out_tag)
            k = next(gen)
            while True:
                r = gen.send(pieces[k])
                if not isinstance(r, int):
                    return r
                k = r

        bvec = run_chain(P, 1, [
            (lambda o: wlb[0][:, 0:128], bgc0[:]),
            (lambda o: wlb[1][0:F - P, 0:128], bgc1[0:F - P, :]),
        ], blb, "bvec")

        d1 = run_chain(125, 4, [
            (lambda o: wd1[0][:, o * 125:(o + 1) * 125], mvec[:]),
            (lambda o: wd1[1][:, o * 125:(o + 1) * 125], bvec[:]),
        ], bd1, "d1")

        d2 = run_chain(P, 2, [
            (lambda o, k=k: wd2t[k][:, o * P:(o + 1) * P], d1[:, k:k + 1])
            for k in range(4)
        ], bd2, "d2")

        c1in = sb.tile([P, 8], f32, tag="c1in", bufs=1)
        nc.sync.dma_start(c1in[:], ar_out.ap().rearrange("(j p) -> p j", p=P))
        c1t = sb.tile([P, 8], f32, tag="c1t", bufs=1)
        nc.vector.tensor_tensor(c1t[:], c1in[:], bl1c[:], op=ADD)
        c1h = sb.tile([P, 8], f16, tag="c1h", bufs=1)
        nc.scalar.activation(c1h[:], c1t[:], RELU)

        def rowstage(lhs_cols, rhs_pk, rhs_cw, ncols, bias_row, tag,
                     out_f16=True):
            """out_row[1, ncols(+pad)] = relu(sum_j lhs_cols[j]^T rhs_j + b)."""
            npad = max(ncols, 1024) if ncols > 512 else ncols
            row = sb.tile([1, npad], f32, tag=tag + "r", bufs=1)
            if npad > ncols:
                nc.vector.memset(row[0:1, ncols:npad], 0.0)
            for h0 in range(0, ncols, 500):
                hw = min(500, ncols - h0)
                psr = psM.tile([1, hw], f32, tag="m", space="PSUM")
                for j, col in enumerate(lhs_cols):
                    nc.tensor.matmul(psr[:], col,
                                     rhs_pk[:, j * rhs_cw + h0:
                                            j * rhs_cw + h0 + hw],
                                     start=(j == 0),
                                     stop=(j == len(lhs_cols) - 1))
                nc.vector.tensor_copy(row[0:1, h0:h0 + hw], psr[0:1, :])
            rb = sb.tile([1, npad], f32, tag=tag + "b", bufs=1)
            nc.vector.tensor_tensor(rb[0:1, :], row[0:1, :],
                                    bias_row[0:1, 0:npad], op=ADD)
            ro = sb.tile([1, npad], f16 if out_f16 else f32, tag=tag + "o",
                         bufs=1)
            nc.scalar.activation(ro[0:1, :], rb[0:1, :], RELU)
            return ro

        def rowcols(row, n, tag, idf):
            cols = sb.tile([P, n], row.dtype, tag=tag, bufs=1)
            for j in range(n):
                ptj = psT.tile([P, P], row.dtype, tag="tp", name="ptj",
                               space="PSUM")
                nc.tensor.transpose(ptj[0:P, 0:1], row[0:1, j * P:(j + 1) * P],
                                    idf[0:1, 0:1])
                nc.vector.tensor_copy(cols[:, j:j + 1], ptj[0:P, 0:1])
            return cols

        c1cols = [c1h[:, j:j + 1] for j in range(8)]
        c2h = rowstage(c1cols, wl2cpk, 1000, 1000, bl2c, "c2")
        c2c = rowcols(c2h, 8, "c2c", ident16)
        c3h = rowstage([c2c[:, j:j + 1] for j in range(8)], wl3cpk, 256, 256,
                       bl3c, "c3")
        d2h = sb.tile([P, 2], f16, tag="d2h", bufs=1)
        nc.vector.tensor_copy(d2h[:], d2[:])
        c3c = rowcols(c3h, 2, "c3c", ident16)
        ucols_in = [d2h[:, 0:1], d2h[:, 1:2], c3c[:, 0:1], c3c[:, 1:2]]
        uact = rowstage(ucols_in, wcat1pk, 1000, 1000, bcat1, "u",
                        out_f16=False)
        ucols = rowcols(uact, 8, "ucols", ident32)
        pso = psM.tile([1, 1], f32, tag="m", space="PSUM")
        for k in range(8):
            nc.tensor.matmul(pso[:], ucols[:, k:k + 1], wcat2pk[:, k:k + 1],
                             start=(k == 0), stop=(k == 7))
        osb = sb.tile([1, 1], f32, tag="osb", bufs=1)
        nc.scalar.activation(osb[:], pso[:], RELU, bias=bcat2[:])
        nc.sync.dma_start(out[0:1, 0:1], osb[:])


# ------------------------------------------------------------------- entry

_CACHE = {}


def kernel(**inputs):
    in_maps, meta = prep_inputs(inputs)
    if "nc" not in _CACHE:
        _CACHE["nc"] = build_program(meta)
    nc = _CACHE["nc"]
    res = run_bass_kernel_spmd(nc, in_maps, core_ids=list(range(NCORES)))
    return np.asarray(res.results[0]["out"], np.float32)
